# revision 1
# baseline (speedup 1.0000x reference)
"""ContextualRoIAlign Trainium2 kernel.

Problem (hardcoded): B=2, C=256, H=W=56, N=64 boxes, M=8 gt boxes, P=7.
out[b,n,c,p,q] = roi_align(fm[b], box_n)[c,p,q]
                 + mean_m roi_align(fm[b], union(box_n, gt_m))[c,p,q]

Decomposition: roi_align separates per axis into small interpolation
matrices Ay, Ax ([R,7,56], host-precomputed exactly like the reference):
  out[r,c,p,q] = sum_h sum_w Ay[r,p,h] * fm[c,h,w] * Ax[r,q,w]
The 1/M mean weight is folded into Ax of the context rois, and the 9-roi
group sum is accumulated in PSUM.

Sharding: 8 cores; core k handles image k//4, box groups [16*(k%4), +16)
=> 144 rois per core (16 groups x (1 box + 8 ctx)). fm replicated per
image (4 cores each).

Device program per core (all fp32 by default):
  Stage 1 (contract h): weights = fm channel-pair [h=56, 128] where col
    c_loc*64+w holds fm[2i+c_loc, h, w]; rhs = AyT [56, 504] (all rois'
    (r,p) columns, 2 chunks); psum [128, 504] -> TMP[128, i_loc, 1008].
    TMP partition psi*64+w holds tmp[c=2i+psi, w, r, p]: w ends up on
    partitions with no inter-stage transpose.
  Stage 2 (contract w): weights = TMP[psi*64:+56, :, r*7:+7] ([56,16,7],
    M=112=(c_i,p)); rhs = AxT[psi*64:+56, r*7:+7] ([56,7]); 9 rois of a
    group accumulate into one psum [112,7] = final out chunk.
"""
import os
import numpy as np

P = 7
B, C, H, W, N, M = 2, 256, 56, 56, 64, 8
NCORES = 8
GROUPS_PER_CORE = 16
ROIS_PER_GROUP = 9
R_CORE = GROUPS_PER_CORE * ROIS_PER_GROUP   # 144
RP = R_CORE * P                              # 1008
NPAIRS = 128
WIN = 16                                     # channel pairs per window
NWIN = NPAIRS // WIN                         # 8
NCHUNK = 504                                 # stage-1 rhs cols per matmul


# ---------------------------------------------------------------- host prep

def _axis_weights(start, length, dim):
    """Exact numpy port of the reference's _axis_weights (float32)."""
    start = start.astype(np.float32)
    length = length.astype(np.float32)
    R = start.shape[0]
    S = int(np.ceil(dim / P))
    bin_sz = length / np.float32(P)
    grid = np.ceil(length / np.float32(P)).astype(np.int32)
    g = grid.astype(np.float32)[:, None, None]
    s = np.arange(S, dtype=np.float32)
    ph = np.arange(P, dtype=np.float32)
    coord = (start[:, None, None] + ph[None, :, None] * bin_sz[:, None, None]
             + (s[None, None, :] + np.float32(0.5)) * bin_sz[:, None, None] / g)
    valid = (coord >= -1.0) & (coord <= dim)
    c = np.maximum(coord, np.float32(0.0))
    low = np.floor(c).astype(np.int32)
    hi_clamp = low >= dim - 1
    low = np.where(hi_clamp, dim - 1, low)
    high = np.where(hi_clamp, dim - 1, low + 1)
    cv = np.where(hi_clamp, low.astype(np.float32), c)
    l = cv - low.astype(np.float32)
    smask = (s[None, None, :] < g) & valid
    w = smask.astype(np.float32) / g
    w_low = ((np.float32(1.0) - l) * w).astype(np.float32)
    w_high = (l * w).astype(np.float32)
    A = np.zeros((R, P, dim), dtype=np.float32)
    r_idx = np.broadcast_to(np.arange(R)[:, None, None], low.shape)
    p_idx = np.broadcast_to(np.arange(P)[None, :, None], low.shape)
    np.add.at(A, (r_idx, p_idx, low), w_low)
    np.add.at(A, (r_idx, p_idx, high), w_high)
    return A


def _prep_core(fm_b, boxes_b, gt_b, g0):
    b = boxes_b.astype(np.float32)
    g = gt_b.astype(np.float32)
    x1 = np.minimum(b[:, None, 0], g[None, :, 0])
    y1 = np.minimum(b[:, None, 1], g[None, :, 1])
    x2 = np.maximum(b[:, None, 2], g[None, :, 2])
    y2 = np.maximum(b[:, None, 3], g[None, :, 3])
    ctx = np.stack([x1, y1, x2, y2], axis=-1)                 # [N,M,4]
    rois = np.concatenate([b[:, None, :], ctx], axis=1)       # [N,9,4]
    wts = np.full((N, ROIS_PER_GROUP), np.float32(1.0 / M), dtype=np.float32)
    wts[:, 0] = np.float32(1.0)

    rois = rois[g0:g0 + GROUPS_PER_CORE].reshape(R_CORE, 4)
    wts = wts[g0:g0 + GROUPS_PER_CORE].reshape(R_CORE)
    x1, y1, x2, y2 = rois[:, 0], rois[:, 1], rois[:, 2], rois[:, 3]
    roi_w = np.maximum(x2 - x1, np.float32(1.0))
    roi_h = np.maximum(y2 - y1, np.float32(1.0))
    Ay = _axis_weights(y1, roi_h, H)
    Ax = _axis_weights(x1, roi_w, W) * wts[:, None, None]

    AyT = np.ascontiguousarray(Ay.transpose(2, 0, 1).reshape(H, RP))
    # Ax with q padded 7->8 (fp32r matmuls need even free sizes)
    Ax8 = np.zeros((R_CORE, 8, W), dtype=np.float32)
    Ax8[:, :P] = Ax
    AxT = Ax8.transpose(2, 0, 1).reshape(W, R_CORE * 8)
    AxT_dup = np.zeros((128, R_CORE * 8), dtype=np.float32)
    AxT_dup[0:56] = AxT
    AxT_dup[64:120] = AxT

    F3 = np.zeros((H, NPAIRS, 128), dtype=np.float32)
    fmT = fm_b.transpose(1, 0, 2)                              # [h, c, w]
    F3[:, :, 0:56] = fmT[:, 0::2, :]
    F3[:, :, 64:120] = fmT[:, 1::2, :]
    return np.ascontiguousarray(F3), AyT, np.ascontiguousarray(AxT_dup)


def _unpack_core_out(OUT):
    """OUT [112,16,112] -> [16, 256, 7, 7]."""
    a = OUT.reshape(WIN, P, GROUPS_PER_CORE, 2, NWIN, P)
    a = a.transpose(2, 4, 0, 3, 1, 5)            # [g, win, c_i, psi, p, q]
    return np.ascontiguousarray(a.reshape(GROUPS_PER_CORE, C, P, P))


# ---------------------------------------------------------------- program

_PROGRAM = None


def _build_program():
    import concourse.bacc as bacc
    import concourse.tile as tile
    import concourse.mybir as mybir

    f32 = mybir.dt.float32
    dts = {"float32": mybir.dt.float32, "float32r": mybir.dt.float32r,
           "bfloat16": mybir.dt.bfloat16}
    s1_dt = dts[os.environ.get("ROI_S1_DTYPE", "float32r")]
    s2_dt = dts[os.environ.get("ROI_S2_DTYPE", "float32r")]

    nc = bacc.Bacc("TRN2", target_bir_lowering=False, debug=False,
                   enable_asserts=False)
    f3_d = nc.dram_tensor("f3", [H, NPAIRS, 128], f32, kind="ExternalInput").ap()
    ayt_d = nc.dram_tensor("ayt", [H, RP], f32, kind="ExternalInput").ap()
    axt_d = nc.dram_tensor("axt", [128, R_CORE * 8], f32, kind="ExternalInput").ap()
    out_d = nc.dram_tensor("out", [112, GROUPS_PER_CORE, 112], f32,
                           kind="ExternalOutput").ap()

    with tile.TileContext(nc) as tc:
        with tc.tile_pool(name="const", bufs=1) as cpool, \
             tc.tile_pool(name="fmw", bufs=2) as fpool, \
             tc.tile_pool(name="tmp", bufs=2) as tpool, \
             tc.tile_pool(name="outp", bufs=1) as opool, \
             tc.tile_pool(name="ps1", bufs=3, space="PSUM") as ps1p, \
             tc.tile_pool(name="ps2", bufs=4, space="PSUM") as ps2p:

            AyT_raw = cpool.tile([H, RP], f32)
            nc.sync.dma_start(AyT_raw[:], ayt_d)
            AxT_raw = cpool.tile([128, R_CORE * 8], f32)
            nc.sync.dma_start(AxT_raw[:], axt_d)
            if s1_dt != f32:
                AyT = cpool.tile([H, RP], s1_dt)
                nc.vector.tensor_copy(out=AyT[:], in_=AyT_raw[:])
            else:
                AyT = AyT_raw
            if s2_dt != f32:
                AxT = cpool.tile([128, R_CORE * 8], s2_dt)
                nc.vector.tensor_copy(out=AxT[:], in_=AxT_raw[:])
            else:
                AxT = AxT_raw
            OUT = opool.tile([112, GROUPS_PER_CORE, 112], f32)

            ncopy = 0
            for win in range(NWIN):
                F3raw = fpool.tile([H, WIN, 128], f32, tag="f3raw")
                nc.sync.dma_start(F3raw[:], f3_d[:, win * WIN:(win + 1) * WIN, :])
                if s1_dt != f32:
                    F3w = fpool.tile([H, WIN, 128], s1_dt, tag="f3w")
                    nc.scalar.copy(out=F3w[:], in_=F3raw[:])
                else:
                    F3w = F3raw
                # TMP[psi*64+w, r, c_i*7+p] = tmp[c=2*(win*16+c_i)+psi, w, r, p]
                # (layout r-major so a stage-2 weights slice is one
                # contiguous 112-element free dim)
                TMP = tpool.tile([128, R_CORE, WIN * P], s2_dt, tag="tmp")
                for il in range(WIN):
                    for ch in range(2):
                        ps = ps1p.tile([128, NCHUNK], f32, tag="ps1")
                        nc.tensor.matmul(
                            ps[:],
                            F3w[:, il, :],
                            AyT[:, ch * NCHUNK:(ch + 1) * NCHUNK],
                            start=True, stop=True)
                        dst = TMP[:, ch * 72:(ch + 1) * 72, il * P:(il + 1) * P]
                        if ncopy % 2 == 0:
                            nc.vector.tensor_copy(out=dst, in_=ps[:])
                        else:
                            nc.scalar.copy(out=dst, in_=ps[:])
                        ncopy += 1
                for g in range(GROUPS_PER_CORE):
                    for psi in range(2):
                        ps2 = ps2p.tile([112, 8], f32, tag="ps2")
                        for j in range(ROIS_PER_GROUP):
                            r = g * ROIS_PER_GROUP + j
                            lhsT = TMP[psi * 64:psi * 64 + 56, r, :]
                            rhs = AxT[psi * 64:psi * 64 + 56, r * 8:(r + 1) * 8]
                            nc.tensor.matmul(
                                ps2[:], lhsT, rhs,
                                start=(j == 0), stop=(j == ROIS_PER_GROUP - 1))
                        nc.any.tensor_copy(
                            out=OUT[:, g, (psi * NWIN + win) * P:(psi * NWIN + win + 1) * P],
                            in_=ps2[:, 0:P])
            nc.sync.dma_start(out_d, OUT[:])

    nc.compile()
    return nc


LAST_RESULT = None


def _ensure_axon_hooks_shim():
    """concourse's axon trace path imports antenv.axon_hooks, which this
    image's antenv package lacks; provide a minimal registry so a stray
    BASS_TRACE=1 in the environment cannot crash the kernel."""
    try:
        import antenv  # noqa: F401
        import antenv.axon_hooks  # noqa: F401
        return
    except ImportError:
        pass
    try:
        import sys
        import types
        import antenv
        mod = types.ModuleType("antenv.axon_hooks")
        mod._hook = None
        mod.get_axon_ntff_profile_hook = lambda: mod._hook

        def _set(h):
            mod._hook = h

        mod.set_axon_ntff_profile_hook = _set
        sys.modules["antenv.axon_hooks"] = mod
        antenv.axon_hooks = mod
    except Exception:
        pass


def kernel(feature_map, boxes, gt_boxes):
    global _PROGRAM, LAST_RESULT
    _ensure_axon_hooks_shim()
    feature_map = np.asarray(feature_map, dtype=np.float32)
    boxes = np.asarray(boxes, dtype=np.float32)
    gt_boxes = np.asarray(gt_boxes, dtype=np.float32)

    from concourse.bass_utils import run_bass_kernel_spmd

    if _PROGRAM is None:
        _PROGRAM = _build_program()
    nc = _PROGRAM

    in_maps = []
    for k in range(NCORES):
        b = k // 4
        g0 = (k % 4) * GROUPS_PER_CORE
        F3, AyT, AxT_dup = _prep_core(feature_map[b], boxes[b], gt_boxes[b], g0)
        in_maps.append({"f3": F3, "ayt": AyT, "axt": AxT_dup})

    trace = bool(int(os.environ.get("ROI_TRACE", "0")))
    res = run_bass_kernel_spmd(nc, in_maps, list(range(NCORES)), trace=trace)
    LAST_RESULT = res

    out = np.zeros((B, N, C, P, P), dtype=np.float32)
    for k in range(NCORES):
        b = k // 4
        g0 = (k % 4) * GROUPS_PER_CORE
        out[b, g0:g0 + GROUPS_PER_CORE] = _unpack_core_out(res.results[k]["out"])
    return out



# revision 8
# speedup vs baseline: 1.9392x; 1.9392x over previous
"""ContextualRoIAlign Trainium2 kernel (v2: bf16 + streamed stage-2).

Problem (hardcoded): B=2, C=256, H=W=56, N=64 boxes, M=8 gt boxes, P=7.
out[b,n,c,p,q] = roi_align(fm[b], box_n)[c,p,q]
                 + mean_m roi_align(fm[b], union(box_n, gt_m))[c,p,q]

Decomposition: roi_align separates per axis into small interpolation
matrices Ay, Ax ([R,7,56], host-precomputed exactly like the reference):
  out[r,c,p,q] = sum_h Ay[r,p,h] * (sum_w fm[c,h,w] * Ax[r,q,w])
The 1/M mean weight is folded into Ax of the context rois, and the 9-roi
group sum is accumulated in PSUM.

Sharding: 8 cores; core k handles image k//4, box groups [16*(k%4), +16)
=> 144 rois per core (16 groups x (1 box + 8 ctx)). fm replicated per
image (4 cores each).

Device program per core (bf16 matmul inputs, fp32 psum accumulate):
  Stage 1 (contract h): weights = fm channel-pair [h=56, 128] where col
    c_loc*64+w holds fm[2i+c_loc, h, w]; rhs = AyT [56, 504] (72 rois'
    (r,p) columns per chunk); psum [128, 504] -> TMP[128, 72, 112] bf16.
    TMP partition c_loc*64+w holds tmp[c=2*(win*16+il)+c_loc, w] at free
    col (r_loc, il*7+p): w on partitions, no inter-stage transpose.
  Stage 2 (contract w): stationary = AxBD[:, r*16:+16] ([128,16] block-
    diagonal: rows 0:56 have Ax[r] in cols 0:8, rows 64:120 have Ax[r]
    in cols 8:16 => the 16-col LDWEIGHTS hides under the previous
    matmul); moving = TMP[:, r_loc, :] ([128, 112]); out [16,112].
    9 rois of a group accumulate in one psum chain; 4 groups share one
    psum tile at PE col-tile offsets 0/32/64/96.
"""
import os
import numpy as np
import ml_dtypes

P = 7
B, C, H, W, N, M = 2, 256, 56, 56, 64, 8
NCORES = 8
GROUPS_PER_CORE = 16
ROIS_PER_GROUP = 9
R_CORE = GROUPS_PER_CORE * ROIS_PER_GROUP   # 144
RP = R_CORE * P                              # 1008
WIN = 16                                     # channel pairs per window
NWIN = 128 // WIN                            # 8
NCHUNK = 504                                 # stage-1 rhs cols per matmul
RCHUNK = NCHUNK // P                         # 72 rois per TMP tile

BF16 = ml_dtypes.bfloat16


# ---------------------------------------------------------------- host prep

def _axis_weights(start, length, dim):
    """Exact numpy port of the reference's _axis_weights (float32)."""
    start = start.astype(np.float32)
    length = length.astype(np.float32)
    R = start.shape[0]
    S = int(np.ceil(dim / P))
    bin_sz = length / np.float32(P)
    grid = np.ceil(length / np.float32(P)).astype(np.int32)
    g = grid.astype(np.float32)[:, None, None]
    s = np.arange(S, dtype=np.float32)
    ph = np.arange(P, dtype=np.float32)
    coord = (start[:, None, None] + ph[None, :, None] * bin_sz[:, None, None]
             + (s[None, None, :] + np.float32(0.5)) * bin_sz[:, None, None] / g)
    valid = (coord >= -1.0) & (coord <= dim)
    c = np.maximum(coord, np.float32(0.0))
    low = np.floor(c).astype(np.int32)
    hi_clamp = low >= dim - 1
    low = np.where(hi_clamp, dim - 1, low)
    high = np.where(hi_clamp, dim - 1, low + 1)
    cv = np.where(hi_clamp, low.astype(np.float32), c)
    l = cv - low.astype(np.float32)
    smask = (s[None, None, :] < g) & valid
    w = smask.astype(np.float32) / g
    w_low = ((np.float32(1.0) - l) * w).astype(np.float32)
    w_high = (l * w).astype(np.float32)
    A = np.zeros((R, P, dim), dtype=np.float32)
    r_idx = np.broadcast_to(np.arange(R)[:, None, None], low.shape)
    p_idx = np.broadcast_to(np.arange(P)[None, :, None], low.shape)
    np.add.at(A, (r_idx, p_idx, low), w_low)
    np.add.at(A, (r_idx, p_idx, high), w_high)
    return A


def _prep_core(fm_b, boxes_b, gt_b, g0):
    b = boxes_b.astype(np.float32)
    g = gt_b.astype(np.float32)
    x1 = np.minimum(b[:, None, 0], g[None, :, 0])
    y1 = np.minimum(b[:, None, 1], g[None, :, 1])
    x2 = np.maximum(b[:, None, 2], g[None, :, 2])
    y2 = np.maximum(b[:, None, 3], g[None, :, 3])
    ctx = np.stack([x1, y1, x2, y2], axis=-1)                 # [N,M,4]
    rois = np.concatenate([b[:, None, :], ctx], axis=1)       # [N,9,4]
    wts = np.full((N, ROIS_PER_GROUP), np.float32(1.0 / M), dtype=np.float32)
    wts[:, 0] = np.float32(1.0)

    rois = rois[g0:g0 + GROUPS_PER_CORE].reshape(R_CORE, 4)
    wts = wts[g0:g0 + GROUPS_PER_CORE].reshape(R_CORE)
    x1, y1, x2, y2 = rois[:, 0], rois[:, 1], rois[:, 2], rois[:, 3]
    roi_w = np.maximum(x2 - x1, np.float32(1.0))
    roi_h = np.maximum(y2 - y1, np.float32(1.0))
    Ay = _axis_weights(y1, roi_h, H)                          # [R,P,H]
    Ax = _axis_weights(x1, roi_w, W) * wts[:, None, None]     # [R,P,W]

    AyT = np.ascontiguousarray(Ay.transpose(2, 0, 1).reshape(H, RP))
    # AxBD [128, R*16] block-diagonal per roi: rows w hold Ax[r,q,w] at
    # col r*16+q; rows 64+w hold the same at col r*16+8+q.
    AxBD = np.zeros((128, R_CORE * 16), dtype=np.float32)
    AxT = Ax.transpose(2, 0, 1)                               # [W, R, P]
    for psi in range(2):
        blk = AxBD[psi * 64:psi * 64 + W].reshape(W, R_CORE, 16)
        blk[:, :, psi * 8:psi * 8 + P] = AxT

    F3 = np.zeros((H, 128, 128), dtype=np.float32)
    fmT = fm_b.transpose(1, 0, 2)                              # [h, c, w]
    F3[:, :, 0:56] = fmT[:, 0::2, :]
    F3[:, :, 64:120] = fmT[:, 1::2, :]
    return (F3.astype(BF16), AyT.astype(BF16), AxBD.astype(BF16))


def _unpack_core_out(OUT):
    """OUT [2, 16, 8, 8, 112] -> [16, 256, 7, 7].

    OUT[kk, psi*8 + q, gb, win, il*7 + p] = out[g=2*gb+kk,
    c=2*(win*16+il)+psi, p, q].
    """
    a = OUT.reshape(2, 2, 8, 8, NWIN, WIN, P)[:, :, :P]   # [kk,psi,q,gb,win,il,p]
    a = a.transpose(3, 0, 4, 5, 1, 6, 2)                  # [gb,kk,win,il,psi,p,q]
    return np.ascontiguousarray(a.reshape(GROUPS_PER_CORE, C, P, P))


# ---------------------------------------------------------------- program

_PROGRAM = None


def _build_program():
    import concourse.bacc as bacc
    import concourse.tile as tile
    import concourse.mybir as mybir

    f32 = mybir.dt.float32
    bf16 = mybir.dt.bfloat16

    nc = bacc.Bacc("TRN2", target_bir_lowering=False, debug=False,
                   enable_asserts=False)
    f3_d = nc.dram_tensor("f3", [H, 128, 128], bf16, kind="ExternalInput").ap()
    ayt_d = nc.dram_tensor("ayt", [H, RP], bf16, kind="ExternalInput").ap()
    axbd_d = nc.dram_tensor("axbd", [128, R_CORE * 16], bf16,
                            kind="ExternalInput").ap()
    out_d = nc.dram_tensor("out", [2, 16, 8, NWIN, 112], f32,
                           kind="ExternalOutput").ap()

    with tile.TileContext(nc) as tc:
        with tc.tile_pool(name="const", bufs=1) as cpool, \
             tc.tile_pool(name="fmw", bufs=2) as fpool, \
             tc.tile_pool(name="tmp", bufs=2) as tpool, \
             tc.tile_pool(name="outp", bufs=1) as opool, \
             tc.tile_pool(name="ps1", bufs=4, space="PSUM") as ps1p, \
             tc.tile_pool(name="ps2", bufs=4, space="PSUM") as ps2p:

            AyT = cpool.tile([H, RP], bf16)
            nc.sync.dma_start(AyT[:], ayt_d)
            AxBD = cpool.tile([128, R_CORE * 16], bf16)
            nc.sync.dma_start(AxBD[:], axbd_d)
            OUT = opool.tile([128, 8, NWIN, 112], f32)

            ncopy = 0
            for win in range(NWIN):
                F3w = fpool.tile([H, WIN, 128], bf16, tag="f3w")
                nc.sync.dma_start(F3w[:], f3_d[:, win * WIN:(win + 1) * WIN, :])
                tmps = []
                for ch in range(2):
                    TMP = tpool.tile([128, RCHUNK, WIN * P], bf16,
                                     tag=f"tmp{ch}")
                    tmps.append(TMP)
                    for il in range(WIN):
                        ps = ps1p.tile([128, NCHUNK], f32, tag="ps1")
                        nc.tensor.matmul(
                            ps[:],
                            F3w[:, il, :],
                            AyT[:, ch * NCHUNK:(ch + 1) * NCHUNK],
                            start=True, stop=True)
                        dst = TMP[:, :, il * P:(il + 1) * P]
                        if ncopy % 2 == 0:
                            nc.vector.tensor_copy(out=dst, in_=ps[:])
                        else:
                            nc.scalar.copy(out=dst, in_=ps[:])
                        ncopy += 1
                for gb in range(8):
                    ps2 = ps2p.tile([128, 112], f32, tag="ps2")
                    for k in range(2):
                        g = gb * 2 + k
                        for j in range(ROIS_PER_GROUP):
                            r = g * ROIS_PER_GROUP + j
                            ch, rloc = divmod(r, RCHUNK)
                            nc.tensor.matmul(
                                ps2[64 * k:64 * k + 16, :],
                                AxBD[:, r * 16:(r + 1) * 16],
                                tmps[ch][:, rloc, :],
                                start=(j == 0), stop=(j == ROIS_PER_GROUP - 1))
                    nc.vector.tensor_copy(out=OUT[:, gb, win, :], in_=ps2[:])
            nc.sync.dma_start(out_d[0], OUT[0:16])
            nc.sync.dma_start(out_d[1], OUT[64:80])

    nc.compile()
    return nc


LAST_RESULT = None


def _ensure_axon_hooks_shim():
    """concourse's axon trace path imports antenv.axon_hooks, which this
    image's antenv package lacks; provide a minimal registry so a stray
    BASS_TRACE=1 in the environment cannot crash the kernel."""
    try:
        import antenv  # noqa: F401
        import antenv.axon_hooks  # noqa: F401
        return
    except ImportError:
        pass
    try:
        import sys
        import types
        import antenv
        mod = types.ModuleType("antenv.axon_hooks")
        mod._hook = None
        mod.get_axon_ntff_profile_hook = lambda: mod._hook

        def _set(h):
            mod._hook = h

        mod.set_axon_ntff_profile_hook = _set
        sys.modules["antenv.axon_hooks"] = mod
        antenv.axon_hooks = mod
    except Exception:
        pass


def kernel(feature_map, boxes, gt_boxes):
    global _PROGRAM, LAST_RESULT
    _ensure_axon_hooks_shim()
    feature_map = np.asarray(feature_map, dtype=np.float32)
    boxes = np.asarray(boxes, dtype=np.float32)
    gt_boxes = np.asarray(gt_boxes, dtype=np.float32)

    from concourse.bass_utils import run_bass_kernel_spmd

    if _PROGRAM is None:
        _PROGRAM = _build_program()
    nc = _PROGRAM

    in_maps = []
    for k in range(NCORES):
        b = k // 4
        g0 = (k % 4) * GROUPS_PER_CORE
        F3, AyT, AxBD = _prep_core(feature_map[b], boxes[b], gt_boxes[b], g0)
        in_maps.append({"f3": F3, "ayt": AyT, "axbd": AxBD})

    trace = bool(int(os.environ.get("ROI_TRACE", "0")))
    res = run_bass_kernel_spmd(nc, in_maps, list(range(NCORES)), trace=trace)
    LAST_RESULT = res

    out = np.zeros((B, N, C, P, P), dtype=np.float32)
    for k in range(NCORES):
        b = k // 4
        g0 = (k % 4) * GROUPS_PER_CORE
        out[b, g0:g0 + GROUPS_PER_CORE] = _unpack_core_out(res.results[k]["out"])
    return out
